# revision 2
# baseline (speedup 1.0000x reference)
"""GAT layer (PyG GATConv-style, single head) on 8 Trainium2 NeuronCores.

Strategy: dst-sharded edge parallelism.
  - Host (index-only prep): append self-loops, sort edges by destination,
    give each core a contiguous range of 6250 destination nodes.  The
    scatter-softmax segments are then fully core-local -> no collectives.
  - Pass 0 (per core): h = x @ W via PE (bf16), plus a_s = h@att_src and
    a_d = h@att_dst folded in as two extra columns of the weight matrix
    (w_s = W@att_src computed on device).  Writes an augmented row table
    hA[n] = [a_s fp32 | a_d fp32 | 1.0 | h bf16*256 | pad] (768B rows).
  - Pass 1 (per core): for each 128-dst window, dma_gather the edge rows.
    Per-slot a_d[dst] comes from a second tiny gather: the window's a_d
    column (extracted from the partition-aligned self-loop rows) is
    broadcast to a [128,64] tile, written to a per-window DRAM mini-table
    (32KB), and gathered back with idx = window-local dst (0..127) --
    this lands a_d[dst_of_slot] directly at [slot_partition, tile] with
    no per-tile compute.  e = leakyrelu(a_s + a_d) and exp are then
    window-level DVE/ACT ops; one dual-op tensor_scalar per tile builds
    the one-hot(dst)*exp selection matrix, and matmul-accumulates
    S.T @ [1|h] into PSUM.  The ones column produces the softmax
    denominator in PSUM column 0; the epilogue multiplies by its
    reciprocal.
  - No max-subtraction in the softmax: inputs are gaussian so |e| < ~15 and
    fp32 exp cannot overflow; alpha is mathematically identical.

dma_gather requires int16 indices, so the node table is split at 32768
(lo/hi tables); window edges are grouped lo-first, hi-second, each group
padded to a multiple of 128 (pad edges gather row 0 and carry a local-dst
id of 255 so their one-hot row is all zero -> no contribution).
"""

import os
import sys

sys.path.insert(0, "/opt/trn_rl_repo")

import numpy as np
import ml_dtypes

P = 128
C = 256  # in_c == out_c
GCOLS = 384  # bf16 columns per hA row (768B, must be a multiple of 128 cols)
RHS_OFF = 4  # rhs slice = cols [4, 4+257): [1.0 | h*256]
PAD_DLOC = 255.0  # local-dst id for pad edges (never matches iota 0..127)

_BF16 = ml_dtypes.bfloat16

TRACE = False
TRACE_ALL_CORES = True
_CACHE = {}


# --------------------------------------------------------------------------
# Host-side prep: pure index manipulation (sharding / layout), no float math
# --------------------------------------------------------------------------
def _prep_edges(edge_index, n_nodes, n_cores, split):
    src = np.asarray(edge_index[0], dtype=np.int64)
    dst = np.asarray(edge_index[1], dtype=np.int64)
    loops = np.arange(n_nodes, dtype=np.int64)
    src = np.concatenate([src, loops])
    dst = np.concatenate([dst, loops])

    d_per_core = n_nodes // n_cores
    nw = (d_per_core + P - 1) // P  # windows (128 dsts) per core

    core = dst // d_per_core
    ldst = dst - core * d_per_core
    win = ldst // P
    dloc = ldst % P
    is_hi = (src >= split).astype(np.int64)

    key = (core * nw + win) * 2 + is_hi
    order = np.argsort(key, kind="stable")
    src_s = src[order]
    key_s = key[order]
    core_s = core[order]
    win_s = win[order]
    dloc_s = dloc[order]
    hi_s = is_hi[order]

    is_loop = np.zeros(src.size, dtype=bool)
    is_loop[-n_nodes:] = True
    loop_s = is_loop[order]

    cnt = np.bincount(key_s, minlength=n_cores * nw * 2).reshape(n_cores, nw, 2)
    # tiles per window half, shared across cores (SPMD: one program).
    # At least one tile per half: the advec extraction needs tiles 0 and t_lo.
    t_lo = np.maximum(1, np.ceil(cnt[:, :, 0].max(axis=0) / P)).astype(np.int64)
    t_hi = np.maximum(1, np.ceil(cnt[:, :, 1].max(axis=0) / P)).astype(np.int64)
    tt = t_lo + t_hi
    tmax = int(tt.max())
    smax = 8 * tmax  # int16 idx columns (16-partition wrap)

    # Slot assignment per (core, win):
    #  - the self-loop of window-dst p goes to slot p (if lo) or t_lo*P+p (if
    #    hi), so tile 0 / tile t_lo carry a_d[dst] partition-aligned;
    #  - remaining edges fill the remaining slots of their half in order.
    widx = np.zeros((n_cores, nw, P, smax), dtype=np.int16)
    widx2 = np.zeros((n_cores, nw, P, smax), dtype=np.int16)
    wdl = np.full((n_cores, nw, P, tmax), PAD_DLOC, dtype=np.float32)
    wmask = np.zeros((n_cores, nw, P, 1), dtype=np.float32)

    idx16_s = (src_s - hi_s * split).astype(np.int16)
    starts = np.zeros(n_cores * nw * 2 + 1, dtype=np.int64)
    np.cumsum(cnt.reshape(-1), out=starts[1:])

    for k in range(n_cores):
        for w in range(nw):
            tl = int(t_lo[w])
            cap = [tl * P, int(t_hi[w]) * P]
            base = [0, tl * P]
            slot_all = np.empty(0, dtype=np.int64)
            eidx_all = np.empty(0, dtype=np.int64)
            for half in range(2):
                g = (k * nw + w) * 2 + half
                lo_i, hi_i = starts[g], starts[g + 1]
                seg = np.arange(lo_i, hi_i)
                loops = loop_s[seg]
                loop_slots = dloc_s[seg[loops]]
                reserved = np.zeros(cap[half], dtype=bool)
                reserved[loop_slots] = True
                free_slots = np.flatnonzero(~reserved)
                nl = seg[~loops]
                slots = np.concatenate([loop_slots, free_slots[: nl.size]])
                eids = np.concatenate([seg[loops], nl])
                slot_all = np.concatenate([slot_all, slots + base[half]])
                eidx_all = np.concatenate([eidx_all, eids])
            widx[k, w, slot_all % 16, slot_all // 16] = idx16_s[eidx_all]
            widx2[k, w, slot_all % 16, slot_all // 16] = dloc_s[eidx_all].astype(
                np.int16
            )
            wdl[k, w, slot_all % P, slot_all // P] = dloc_s[eidx_all].astype(
                np.float32
            )
            wbase = k * d_per_core + w * P
            ndst = min(P, d_per_core - w * P)
            pp = np.arange(ndst)
            wmask[k, w, pp, 0] = (wbase + pp < split).astype(np.float32)

    widx = np.tile(widx[:, :, 0:16, :], (1, 1, 8, 1))
    widx2 = np.tile(widx2[:, :, 0:16, :], (1, 1, 8, 1))
    return (
        widx,
        widx2,
        wdl,
        wmask,
        [int(v) for v in t_lo],
        [int(v) for v in t_hi],
        d_per_core,
        nw,
        tmax,
        smax,
    )


# --------------------------------------------------------------------------
# Device program (identical for all cores; per-core data differs)
# --------------------------------------------------------------------------
def _build_nc(n_nodes, split, d_per_core, t_lo, t_hi, tmax, smax):
    from concourse import bacc, bass, mybir, tile
    from concourse.masks import make_identity

    f32 = mybir.dt.float32
    bf16 = mybir.dt.bfloat16
    i16 = mybir.dt.int16
    i32 = mybir.dt.int32
    AF = mybir.ActivationFunctionType
    OP = mybir.AluOpType

    nw = len(t_lo)
    n_hi = n_nodes - split
    kh_n = C // P  # contraction halves (2)

    nc = bacc.Bacc("TRN2", target_bir_lowering=False, debug=False)

    xT = nc.dram_tensor("xT", [C, n_nodes], f32, kind="ExternalInput")
    Wd = nc.dram_tensor("W", [C, C], f32, kind="ExternalInput")
    att2 = nc.dram_tensor("att2", [C, 2], f32, kind="ExternalInput")
    biasd = nc.dram_tensor("bias", [1, C], f32, kind="ExternalInput")
    widx = nc.dram_tensor("widx", [nw, P, smax], i16, kind="ExternalInput")
    widx2 = nc.dram_tensor("widx2", [nw, P, smax], i16, kind="ExternalInput")
    wdl = nc.dram_tensor("wdl", [nw, P, tmax], f32, kind="ExternalInput")
    wmask = nc.dram_tensor("wmask", [nw, P, 1], f32, kind="ExternalInput")
    outd = nc.dram_tensor("out", [d_per_core, C], f32, kind="ExternalOutput")

    hA_lo = nc.dram_tensor("hA_lo", [split, GCOLS], bf16)
    hA_hi = nc.dram_tensor("hA_hi", [n_hi, GCOLS], bf16)
    adtab = nc.dram_tensor("adtab", [nw, P, 64], f32)

    with tile.TileContext(nc) as tc:
        # ---------------- pass 0: build hA = [a_s|a_d|1|h] ----------------
        with (
            tc.tile_pool(name="p0c", bufs=1) as cp,
            tc.tile_pool(name="p0ps", bufs=1, space="PSUM") as pp,
            tc.tile_pool(name="p0w", bufs=3) as wp,
        ):
            ident = cp.tile([P, P], f32)
            make_identity(nc, ident[:])
            # bias folded into h (softmax weights sum to 1 per dst)
            bias_bf = cp.tile([1, C + 2], bf16)
            nc.vector.memset(bias_bf[:, C : C + 2], 0.0)
            nc.gpsimd.dma_start(bias_bf[:, 0:C], biasd[:])  # cast f32->bf16
            ones1 = cp.tile([1, P], bf16)
            nc.vector.memset(ones1[:], 1.0)
            W_sb = []
            att_sb = []
            for kh in range(kh_n):
                t = cp.tile([P, C], f32, tag=f"Wsb{kh}", name=f"Wsb{kh}")
                nc.sync.dma_start(t[:], Wd[kh * P : (kh + 1) * P, :])
                W_sb.append(t)
                a = cp.tile([P, 2], f32, tag=f"attsb{kh}", name=f"attsb{kh}")
                nc.sync.dma_start(a[:], att2[kh * P : (kh + 1) * P, :])
                att_sb.append(a)
            # WT via PE transpose (fp32)
            WT_sb = [cp.tile([P, C], f32, tag=f"WTsb{i}", name=f"WTsb{i}") for i in range(kh_n)]
            for oh in range(kh_n):
                for kh in range(kh_n):
                    pt = pp.tile([P, P], f32, tag="ptr", bufs=2)
                    nc.tensor.transpose(
                        pt[:], W_sb[kh][:, oh * P : (oh + 1) * P], ident[:]
                    )
                    nc.vector.tensor_copy(WT_sb[oh][:, kh * P : (kh + 1) * P], pt[:])
            # W_all = bf16([W | w_s | w_d]) with w_{s,d} = W @ att_{src,dst}
            W_all = [cp.tile([P, C + 2], bf16, tag=f"Wall{i}", name=f"Wall{i}") for i in range(kh_n)]
            for ih in range(kh_n):
                pws = pp.tile([P, 2], f32, tag="pws", bufs=1)
                for oh in range(kh_n):
                    nc.tensor.matmul(
                        pws[:],
                        lhsT=WT_sb[oh][:, ih * P : (ih + 1) * P],
                        rhs=att_sb[oh][:],
                        start=(oh == 0),
                        stop=(oh == kh_n - 1),
                    )
                nc.vector.tensor_copy(W_all[ih][:, C : C + 2], pws[:])
                nc.vector.tensor_copy(W_all[ih][:, 0:C], W_sb[ih][:])

            nblk = (n_nodes + P - 1) // P
            SB = 16  # node blocks per x slab
            for sb0 in range(0, nblk, SB):
                sbn = min(SB, nblk - sb0)
                c0 = sb0 * P
                ncols = min(sbn * P, n_nodes - c0)
                xsl = [wp.tile([P, SB * P], bf16, tag=f"xsl{kh}", name=f"xsl{kh}") for kh in range(kh_n)]
                for kh in range(kh_n):
                    # SWDGE cast-DMA fp32 -> bf16
                    nc.gpsimd.dma_start(
                        xsl[kh][:, :ncols], xT[kh * P : (kh + 1) * P, c0 : c0 + ncols]
                    )
                for b in range(sbn):
                    r0 = (sb0 + b) * P
                    m = min(P, n_nodes - r0)
                    ph = pp.tile([P, C + 2], f32, tag="ph", bufs=4)
                    for kh in range(kh_n):
                        nc.tensor.matmul(
                            ph[:m, :],
                            lhsT=xsl[kh][:, b * P : b * P + m],
                            rhs=W_all[kh][:],
                            start=(kh == 0),
                            stop=False,
                        )
                    # += ones^T @ bias : adds bias to the h columns only
                    nc.tensor.matmul(
                        ph[:m, :],
                        lhsT=ones1[:, :m],
                        rhs=bias_bf[:],
                        start=False,
                        stop=True,
                    )
                    hab = wp.tile([P, GCOLS], bf16, tag="hab")
                    nc.vector.tensor_copy(hab[:m, RHS_OFF + 1 : RHS_OFF + 1 + C], ph[:m, 0:C])
                    nc.vector.tensor_copy(hab[:m, 0:4].bitcast(f32), ph[:m, C : C + 2])
                    nc.vector.memset(hab[:m, RHS_OFF : RHS_OFF + 1], 1.0)
                    # zero the pad cols: the gather reads whole 768B rows and
                    # uninitialized HBM reads can fault on HW
                    nc.vector.memset(hab[:m, RHS_OFF + 1 + C : GCOLS], 0.0)
                    if r0 < split:
                        nc.sync.dma_start(hA_lo[r0 : r0 + m, :], hab[:m, :])
                    else:
                        nc.sync.dma_start(
                            hA_hi[r0 - split : r0 - split + m, :], hab[:m, :]
                        )

        # ---------------- pass 1: gather + softmax + aggregate ----------------
        with (
            tc.tile_pool(name="p1c", bufs=1) as c1,
            tc.tile_pool(name="p1g", bufs=2) as gp,
            tc.tile_pool(name="p1s", bufs=4) as sp,
            tc.tile_pool(name="p1m", bufs=3) as mp,
            tc.tile_pool(name="p1ps", bufs=2, space="PSUM") as pp1,
        ):
            iota_i = c1.tile([P, P], i32)
            nc.gpsimd.iota(iota_i[:], pattern=[[1, P]], base=0, channel_multiplier=0)
            iota_f = c1.tile([P, P], f32)
            nc.vector.tensor_copy(iota_f[:], iota_i[:])
            iota_bf = c1.tile([P, P], bf16)
            nc.vector.tensor_copy(iota_bf[:], iota_f[:])

            GCH = 8  # tiles per gather call: <=1024 indices (descriptor ring)
            for w in range(nw):
                tl, th = t_lo[w], t_hi[w]
                tt = tl + th
                G = gp.tile([P, tmax * GCOLS], bf16, tag="G")
                G2 = gp.tile([P, tmax * 64], f32, tag="G2")
                idxt = mp.tile([P, smax], i16, tag="idx")
                nc.sync.dma_start(idxt[:], widx[w])
                idxt2 = mp.tile([P, smax], i16, tag="idx2")
                nc.sync.dma_start(idxt2[:], widx2[w])
                dl = mp.tile([P, tmax], f32, tag="dl")
                nc.sync.dma_start(dl[:], wdl[w])
                mk = mp.tile([P, 1], f32, tag="mk")
                nc.sync.dma_start(mk[:], wmask[w])
                for base, ntl, tab in ((0, tl, hA_lo), (tl, th, hA_hi)):
                    for c0 in range(0, ntl, GCH):
                        cn = min(GCH, ntl - c0)
                        t0g = base + c0
                        nc.gpsimd.dma_gather(
                            G[:, t0g * GCOLS : (t0g + cn) * GCOLS].rearrange(
                                "p (t e) -> p t e", e=GCOLS
                            ),
                            tab[:],
                            idxt[:, 8 * t0g : 8 * (t0g + cn)],
                            cn * P,
                            cn * P,
                            GCOLS,
                        )
                gf = G[:].bitcast(f32).rearrange("p (t c) -> p t c", c=GCOLS // 2)

                # advec[p] = a_d of window dst p, merged from the partition-
                # aligned self-loop rows of tile 0 (lo) / tile tl (hi)
                adv = mp.tile([P, 1], f32, tag="adv")
                nc.vector.tensor_tensor(
                    out=adv[:], in0=gf[:, 0, 1:2], in1=gf[:, tl, 1:2], op=OP.subtract
                )
                nc.vector.tensor_tensor(out=adv[:], in0=adv[:], in1=mk[:], op=OP.mult)
                nc.vector.tensor_tensor(
                    out=adv[:], in0=adv[:], in1=gf[:, tl, 1:2], op=OP.add
                )
                # broadcast to a [128,64] row table and round-trip through a
                # per-window DRAM mini-table: the 256B-row gather below lands
                # a_d[dst_of_slot] at [slot_partition, tile] directly.
                advb = mp.tile([P, 64], f32, tag="advb")
                nc.vector.tensor_copy(advb[:], adv[:, 0:1].to_broadcast([P, 64]))
                nc.sync.dma_start(adtab[w], advb[:])
                for c0 in range(0, tt, GCH):
                    cn = min(GCH, tt - c0)
                    nc.gpsimd.dma_gather(
                        G2[:, c0 * 64 : (c0 + cn) * 64].rearrange(
                            "p (t e) -> p t e", e=64
                        ),
                        adtab[w],
                        idxt2[:, 8 * c0 : 8 * (c0 + cn)],
                        cn * P,
                        cn * P,
                        64,
                    )
                g2f = G2[:].rearrange("p (t e) -> p t e", e=64)

                # e = a_s[src] + a_d[dst], leakyrelu, exp -- window-level ops
                ebuf = mp.tile([P, tmax], f32, tag="e")
                nc.vector.tensor_tensor(
                    out=ebuf[:, :tt].rearrange("p (t o) -> p t o", o=1),
                    in0=gf[:, :tt, 0:1],
                    in1=g2f[:, :tt, 0:1],
                    op=OP.add,
                )
                e2 = mp.tile([P, tmax], f32, tag="e2")
                xb = mp.tile([P, tmax], f32, tag="xb")
                nc.vector.tensor_scalar(
                    out=e2[:, :tt], in0=ebuf[:, :tt], scalar1=0.2, scalar2=None,
                    op0=OP.mult,
                )
                nc.vector.tensor_tensor(
                    out=e2[:, :tt], in0=ebuf[:, :tt], in1=e2[:, :tt], op=OP.max
                )
                nc.scalar.activation(out=xb[:, :tt], in_=e2[:, :tt], func=AF.Exp)

                pw = pp1.tile([P, 1 + C], f32, tag="pw")
                for t in range(tt):
                    S = sp.tile([P, P], bf16, tag="S")
                    nc.vector.tensor_scalar(
                        out=S[:],
                        in0=iota_bf[:],
                        scalar1=dl[:, t : t + 1],
                        scalar2=xb[:, t : t + 1],
                        op0=OP.is_equal,
                        op1=OP.mult,
                    )
                    nc.tensor.matmul(
                        pw[:],
                        lhsT=S[:],
                        rhs=G[:, t * GCOLS + RHS_OFF : t * GCOLS + RHS_OFF + 1 + C],
                        start=(t == 0),
                        stop=(t == tt - 1),
                    )
                rows = min(P, d_per_core - w * P)
                rec = mp.tile([P, 1], f32, tag="rec")
                nc.vector.reciprocal(rec[:], pw[:, 0:1])
                osb = mp.tile([P, C], f32, tag="osb")
                nc.vector.tensor_scalar(
                    out=osb[:], in0=pw[:, 1 : 1 + C], scalar1=rec[:, 0:1],
                    scalar2=None, op0=OP.mult,
                )
                nc.sync.dma_start(outd[w * P : w * P + rows, :], osb[:rows, :])

    nc.compile()
    return nc


# --------------------------------------------------------------------------
# Entry point
# --------------------------------------------------------------------------
def _get_compiled(edge_index, n_nodes, n_cores, split):
    widx, widx2, wdl, wmask, t_lo, t_hi, d_per_core, nw, tmax, smax = _prep_edges(
        edge_index, n_nodes, n_cores, split
    )
    key = (n_nodes, n_cores, split, tuple(t_lo), tuple(t_hi))
    if key not in _CACHE:
        _CACHE[key] = _build_nc(n_nodes, split, d_per_core, t_lo, t_hi, tmax, smax)
    return _CACHE[key], widx, widx2, wdl, wmask, d_per_core


def _in_maps(x, edge_index, W, att_src, att_dst, bias, n_cores, split):
    x = np.asarray(x, dtype=np.float32)
    W = np.asarray(W, dtype=np.float32)
    n_nodes = x.shape[0]
    nc, widx, widx2, wdl, wmask, d_per_core = _get_compiled(
        edge_index, n_nodes, n_cores, split
    )
    xT = np.ascontiguousarray(x.T)
    att2 = np.ascontiguousarray(
        np.stack(
            [np.asarray(att_src, np.float32), np.asarray(att_dst, np.float32)],
            axis=1,
        )
    )
    bias2 = np.ascontiguousarray(np.asarray(bias, np.float32).reshape(1, C))
    in_maps = [
        {
            "xT": xT, "W": W, "att2": att2, "bias": bias2,
            "widx": widx[k], "widx2": widx2[k], "wdl": wdl[k], "wmask": wmask[k],
        }
        for k in range(n_cores)
    ]
    return nc, in_maps, d_per_core


def kernel(x, edge_index, W, att_src, att_dst, bias):
    from concourse.bass_utils import run_bass_kernel_spmd

    n_nodes = np.asarray(x).shape[0]
    n_cores = 8
    split = 32768 if n_nodes > 32768 else max(P, (n_nodes // 2) // P * P)
    nc, in_maps, d_per_core = _in_maps(
        x, edge_index, W, att_src, att_dst, bias, n_cores, split
    )
    kw = {}
    if TRACE:
        kw = dict(trace=True)
        if TRACE_ALL_CORES:
            kw["trace_cores"] = list(range(n_cores))
    res = run_bass_kernel_spmd(nc, in_maps, list(range(n_cores)), **kw)
    out = np.concatenate([res.results[k]["out"] for k in range(n_cores)], axis=0)
    kernel.last_exec_time_ns = res.exec_time_ns
    kernel.last_mean_exec_time_ns = res.mean_exec_time_ns
    return out


kernel.last_exec_time_ns = None
kernel.last_mean_exec_time_ns = None


# --------------------------------------------------------------------------
# Timing helper (no NTFF hook in this environment): time repeated PJRT
# executions with device-resident inputs; subtract a trivial-kernel baseline.
# --------------------------------------------------------------------------
def make_runner(nc, in_maps, n_cores):
    import jax
    import jax.numpy as jnp
    from jax.sharding import Mesh, PartitionSpec
    from jax.experimental.shard_map import shard_map
    from concourse import bass2jax, mybir

    bass2jax.install_neuronx_cc_hook()
    partition_name = (
        nc.partition_id_tensor.name if nc.partition_id_tensor else None
    )
    in_names, out_names, out_avals, zero_outs = [], [], [], []
    for alloc in nc.m.functions[0].allocations:
        if not isinstance(alloc, mybir.MemoryLocationSet):
            continue
        name = alloc.memorylocations[0].name
        if alloc.kind == "ExternalInput":
            if name != partition_name:
                in_names.append(name)
        elif alloc.kind == "ExternalOutput":
            out_names.append(name)
            shape = tuple(alloc.tensor_shape)
            dtype = mybir.dt.np(alloc.dtype)
            out_avals.append(jax.core.ShapedArray(shape, dtype))
            zero_outs.append(np.zeros(shape, dtype))
    n_params = len(in_names)
    all_in_names = list(in_names) + list(out_names)
    if partition_name is not None:
        all_in_names.append(partition_name)

    def _body(*args):
        operands = list(args)
        if partition_name is not None:
            operands.append(bass2jax.partition_id_tensor())
        outs = bass2jax._bass_exec_p.bind(
            *operands,
            out_avals=tuple(out_avals),
            in_names=tuple(all_in_names),
            out_names=tuple(out_names),
            lowering_input_output_aliases=(),
            sim_require_finite=True,
            sim_require_nnan=True,
            nc=nc,
        )
        return tuple(outs)

    devices = jax.devices()[:n_cores]
    mesh = Mesh(np.asarray(devices), ("core",))
    in_specs = (PartitionSpec("core"),) * (n_params + len(out_names))
    out_specs = (PartitionSpec("core"),) * len(out_names)
    fn = jax.jit(
        shard_map(
            _body, mesh=mesh, in_specs=in_specs, out_specs=out_specs,
            check_rep=False,
        ),
        keep_unused=True,
    )
    concat_in = [
        np.concatenate([np.asarray(in_maps[c][nm]) for c in range(n_cores)], axis=0)
        for nm in in_names
    ]
    concat_zeros = [
        np.zeros((n_cores * z.shape[0], *z.shape[1:]), z.dtype) for z in zero_outs
    ]
    sharding = jax.sharding.NamedSharding(mesh, PartitionSpec("core"))
    dev_in = [jax.device_put(a, sharding) for a in concat_in + concat_zeros]

    def run():
        outs = fn(*dev_in)
        jax.block_until_ready(outs)
        return outs

    return run, out_names, out_avals


def timed_kernel(x, edge_index, W, att_src, att_dst, bias, iters=20):
    """Run like kernel() but also time steady-state executions."""
    import time as _time

    n_nodes = np.asarray(x).shape[0]
    n_cores = 8
    split = 32768 if n_nodes > 32768 else max(P, (n_nodes // 2) // P * P)
    nc, in_maps, d_per_core = _in_maps(
        x, edge_index, W, att_src, att_dst, bias, n_cores, split
    )
    run, out_names, out_avals = make_runner(nc, in_maps, n_cores)
    outs = run()  # warmup / compile
    t0 = _time.time()
    for _ in range(iters):
        outs = run()
    dt = (_time.time() - t0) / iters
    oi = out_names.index("out")
    out = np.asarray(outs[oi]).reshape(n_cores, d_per_core, C).reshape(-1, C)
    return out, dt


# revision 22
# speedup vs baseline: 177.8321x; 177.8321x over previous
"""GAT layer (PyG GATConv-style, single head) on 8 Trainium2 NeuronCores.

Strategy: dst-sharded edge parallelism.
  - Host (index-only prep): append self-loops, sort edges by destination,
    give each core a contiguous range of 6250 destination nodes.  The
    scatter-softmax segments are then fully core-local -> no collectives.
  - Pass 0 (per core): h = x @ W via PE (bf16), plus a_s = h@att_src and
    a_d = h@att_dst folded in as two extra columns of the weight matrix
    (w_s = W@att_src computed on device).  Writes an augmented row table
    hA[n] = [a_s fp32 | a_d fp32 | 1.0 | h bf16*256 | pad] (768B rows).
  - Pass 1 (per core): for each 128-dst window, dma_gather the edge rows,
    compute exp(leakyrelu(a_s+a_d)) per edge, build one-hot(dst)*exp
    selection matrices with a single dual-op tensor_scalar, and
    matmul-accumulate S.T @ [1|h] into PSUM.  The ones column produces the
    softmax denominator in PSUM column 0; the epilogue multiplies by its
    reciprocal and adds bias.
  - No max-subtraction in the softmax: inputs are gaussian so |e| < ~15 and
    fp32 exp cannot overflow; alpha is mathematically identical.

dma_gather requires int16 indices, so the node table is split at 32768
(lo/hi tables); window edges are grouped lo-first, hi-second, each group
padded to a multiple of 128 (pad edges gather row 0 and carry a local-dst
id of 255 so their one-hot row is all zero -> no contribution).
"""

import os
import sys

sys.path.insert(0, "/opt/trn_rl_repo")

import numpy as np
import ml_dtypes

P = 128
C = 256  # in_c == out_c
GCOLS = 384  # bf16 columns per hA row (768B, must be a multiple of 128 cols)
RHS_OFF = 4  # rhs slice = cols [4, 4+257): [1.0 | h*256]
PAD_DLOC = 255.0  # local-dst id for pad edges (never matches iota 0..127)

_BF16 = ml_dtypes.bfloat16

TRACE = False
TRACE_ALL_CORES = True
_CACHE = {}


# --------------------------------------------------------------------------
# Host-side prep: pure index manipulation (sharding / layout), no float math
# --------------------------------------------------------------------------
def _prep_edges(edge_index, n_nodes, n_cores, split):
    src = np.asarray(edge_index[0], dtype=np.int64)
    dst = np.asarray(edge_index[1], dtype=np.int64)
    loops = np.arange(n_nodes, dtype=np.int64)
    src = np.concatenate([src, loops])
    dst = np.concatenate([dst, loops])

    d_per_core = n_nodes // n_cores
    nw = (d_per_core + P - 1) // P  # windows (128 dsts) per core

    core = dst // d_per_core
    ldst = dst - core * d_per_core
    win = ldst // P
    dloc = ldst % P
    is_hi = (src >= split).astype(np.int64)

    key = (core * nw + win) * 2 + is_hi
    order = np.argsort(key, kind="stable")
    src_s = src[order]
    key_s = key[order]
    core_s = core[order]
    win_s = win[order]
    dloc_s = dloc[order]
    hi_s = is_hi[order]

    is_loop = np.zeros(src.size, dtype=bool)
    is_loop[-n_nodes:] = True
    loop_s = is_loop[order]

    cnt = np.bincount(key_s, minlength=n_cores * nw * 2).reshape(n_cores, nw, 2)
    # tiles per window half, shared across cores (SPMD: one program).
    # At least one tile per half: the advec extraction needs tiles 0 and t_lo.
    t_lo = np.maximum(1, np.ceil(cnt[:, :, 0].max(axis=0) / P)).astype(np.int64)
    t_hi = np.maximum(1, np.ceil(cnt[:, :, 1].max(axis=0) / P)).astype(np.int64)
    tt = t_lo + t_hi
    tmax = int(tt.max())
    smax = 8 * tmax  # int16 idx columns (16-partition wrap)

    # Slot assignment per (core, win):
    #  - the self-loop of window-dst p goes to slot p (if lo) or t_lo*P+p (if
    #    hi), so tile 0 / tile t_lo carry a_d[dst] partition-aligned;
    #  - remaining edges fill the remaining slots of their half in order.
    widx = np.zeros((n_cores, nw, P, smax), dtype=np.int16)
    wdl = np.full((n_cores, nw, P, tmax), PAD_DLOC, dtype=np.float32)
    wdlr = np.full((n_cores, nw, tmax * P), PAD_DLOC, dtype=np.float32)
    wmask = np.zeros((n_cores, nw, P, 1), dtype=np.float32)

    idx16_s = (src_s - hi_s * split).astype(np.int16)
    starts = np.zeros(n_cores * nw * 2 + 1, dtype=np.int64)
    np.cumsum(cnt.reshape(-1), out=starts[1:])

    for k in range(n_cores):
        for w in range(nw):
            tl = int(t_lo[w])
            cap = [tl * P, int(t_hi[w]) * P]
            base = [0, tl * P]
            slot_all = np.empty(0, dtype=np.int64)
            eidx_all = np.empty(0, dtype=np.int64)
            for half in range(2):
                g = (k * nw + w) * 2 + half
                lo_i, hi_i = starts[g], starts[g + 1]
                seg = np.arange(lo_i, hi_i)
                loops = loop_s[seg]
                loop_slots = dloc_s[seg[loops]]
                reserved = np.zeros(cap[half], dtype=bool)
                reserved[loop_slots] = True
                free_slots = np.flatnonzero(~reserved)
                nl = seg[~loops]
                slots = np.concatenate([loop_slots, free_slots[: nl.size]])
                eids = np.concatenate([seg[loops], nl])
                slot_all = np.concatenate([slot_all, slots + base[half]])
                eidx_all = np.concatenate([eidx_all, eids])
            widx[k, w, slot_all % 16, slot_all // 16] = idx16_s[eidx_all]
            wdl[k, w, slot_all % P, slot_all // P] = dloc_s[eidx_all].astype(
                np.float32
            )
            wdlr[k, w, slot_all] = dloc_s[eidx_all].astype(np.float32)
            wbase = k * d_per_core + w * P
            ndst = min(P, d_per_core - w * P)
            pp = np.arange(ndst)
            wmask[k, w, pp, 0] = (wbase + pp < split).astype(np.float32)

    widx = np.tile(widx[:, :, 0:16, :], (1, 1, 8, 1))
    return (
        widx,
        wdl,
        wdlr.astype(_BF16),
        wmask,
        [int(v) for v in t_lo],
        [int(v) for v in t_hi],
        d_per_core,
        nw,
        tmax,
        smax,
    )


# --------------------------------------------------------------------------
# Device program (identical for all cores; per-core data differs)
# --------------------------------------------------------------------------
def _build_nc(n_nodes, split, d_per_core, t_lo, t_hi, tmax, smax, stage=3, wlim=10**9):
    from concourse import bacc, bass, mybir, tile
    from concourse.masks import make_identity

    f32 = mybir.dt.float32
    bf16 = mybir.dt.bfloat16
    i16 = mybir.dt.int16
    i32 = mybir.dt.int32
    AF = mybir.ActivationFunctionType
    OP = mybir.AluOpType

    nw = len(t_lo)
    n_hi = n_nodes - split
    kh_n = C // P  # contraction halves (2)

    nc = bacc.Bacc("TRN2", target_bir_lowering=False, debug=False)

    xT = nc.dram_tensor("xT", [C, n_nodes], f32, kind="ExternalInput")
    Wd = nc.dram_tensor("W", [C, C], f32, kind="ExternalInput")
    att2 = nc.dram_tensor("att2", [C, 2], f32, kind="ExternalInput")
    biasd = nc.dram_tensor("bias", [1, C], f32, kind="ExternalInput")
    widx = nc.dram_tensor("widx", [nw, P, smax], i16, kind="ExternalInput")
    wdl = nc.dram_tensor("wdl", [nw, P, tmax], f32, kind="ExternalInput")
    wdlr = nc.dram_tensor("wdlr", [nw, 1, tmax * P], bf16, kind="ExternalInput")
    wmask = nc.dram_tensor("wmask", [nw, P, 1], f32, kind="ExternalInput")
    outd = nc.dram_tensor("out", [d_per_core, C], f32, kind="ExternalOutput")

    hA_lo = nc.dram_tensor("hA_lo", [split, GCOLS], bf16)
    hA_hi = nc.dram_tensor("hA_hi", [n_hi, GCOLS], bf16)

    with tile.TileContext(nc) as tc:
        # ---------------- pass 0: build hA = [a_s|a_d|1|h] ----------------
        with (
            tc.tile_pool(name="p0c", bufs=1) as cp,
            tc.tile_pool(name="p0ps", bufs=1, space="PSUM") as pp,
            tc.tile_pool(name="p0w", bufs=3) as wp,
        ):
            ident = cp.tile([P, P], f32)
            make_identity(nc, ident[:])
            # bias folded into h (softmax weights sum to 1 per dst)
            bias_bf = cp.tile([1, C + 2], bf16)
            nc.vector.memset(bias_bf[:, C : C + 2], 0.0)
            nc.gpsimd.dma_start(bias_bf[:, 0:C], biasd[:])  # cast f32->bf16
            ones1 = cp.tile([1, P], bf16)
            nc.vector.memset(ones1[:], 1.0)
            W_sb = []
            att_sb = []
            for kh in range(kh_n):
                t = cp.tile([P, C], f32, tag=f"Wsb{kh}", name=f"Wsb{kh}")
                nc.sync.dma_start(t[:], Wd[kh * P : (kh + 1) * P, :])
                W_sb.append(t)
                a = cp.tile([P, 2], f32, tag=f"attsb{kh}", name=f"attsb{kh}")
                nc.sync.dma_start(a[:], att2[kh * P : (kh + 1) * P, :])
                att_sb.append(a)
            # WT via PE transpose (fp32)
            WT_sb = [cp.tile([P, C], f32, tag=f"WTsb{i}", name=f"WTsb{i}") for i in range(kh_n)]
            for oh in range(kh_n):
                for kh in range(kh_n):
                    pt = pp.tile([P, P], f32, tag="ptr", bufs=2)
                    nc.tensor.transpose(
                        pt[:], W_sb[kh][:, oh * P : (oh + 1) * P], ident[:]
                    )
                    nc.vector.tensor_copy(WT_sb[oh][:, kh * P : (kh + 1) * P], pt[:])
            # W_all = bf16([W | w_s | w_d]) with w_{s,d} = W @ att_{src,dst}
            W_all = [cp.tile([P, C + 2], bf16, tag=f"Wall{i}", name=f"Wall{i}") for i in range(kh_n)]
            for ih in range(kh_n):
                pws = pp.tile([P, 2], f32, tag="pws", bufs=1)
                for oh in range(kh_n):
                    nc.tensor.matmul(
                        pws[:],
                        lhsT=WT_sb[oh][:, ih * P : (ih + 1) * P],
                        rhs=att_sb[oh][:],
                        start=(oh == 0),
                        stop=(oh == kh_n - 1),
                    )
                nc.vector.tensor_copy(W_all[ih][:, C : C + 2], pws[:])
                nc.vector.tensor_copy(W_all[ih][:, 0:C], W_sb[ih][:])

            nblk = (n_nodes + P - 1) // P
            SB = 16  # node blocks per x slab
            for sb0 in range(0, nblk, SB):
                sbn = min(SB, nblk - sb0)
                c0 = sb0 * P
                ncols = min(sbn * P, n_nodes - c0)
                xsl = [wp.tile([P, SB * P], bf16, tag=f"xsl{kh}", name=f"xsl{kh}") for kh in range(kh_n)]
                for kh in range(kh_n):
                    # SWDGE cast-DMA fp32 -> bf16
                    nc.gpsimd.dma_start(
                        xsl[kh][:, :ncols], xT[kh * P : (kh + 1) * P, c0 : c0 + ncols]
                    )
                for b in range(sbn):
                    r0 = (sb0 + b) * P
                    m = min(P, n_nodes - r0)
                    ph = pp.tile([P, C + 2], f32, tag="ph", bufs=4)
                    for kh in range(kh_n):
                        nc.tensor.matmul(
                            ph[:m, :],
                            lhsT=xsl[kh][:, b * P : b * P + m],
                            rhs=W_all[kh][:],
                            start=(kh == 0),
                            stop=False,
                        )
                    # += ones^T @ bias : adds bias to the h columns only
                    nc.tensor.matmul(
                        ph[:m, :],
                        lhsT=ones1[:, :m],
                        rhs=bias_bf[:],
                        start=False,
                        stop=True,
                    )
                    hab = wp.tile([P, GCOLS], bf16, tag="hab")
                    nc.vector.tensor_copy(hab[:m, RHS_OFF + 1 : RHS_OFF + 1 + C], ph[:m, 0:C])
                    nc.vector.tensor_copy(hab[:m, 0:4].bitcast(f32), ph[:m, C : C + 2])
                    nc.vector.memset(hab[:m, RHS_OFF : RHS_OFF + 1], 1.0)
                    # zero the pad cols: the gather reads whole 768B rows and
                    # uninitialized HBM reads can fault on HW
                    nc.vector.memset(hab[:m, RHS_OFF + 1 + C : GCOLS], 0.0)
                    if r0 < split:
                        nc.sync.dma_start(hA_lo[r0 : r0 + m, :], hab[:m, :])
                    else:
                        nc.sync.dma_start(
                            hA_hi[r0 - split : r0 - split + m, :], hab[:m, :]
                        )

        # ---------------- pass 1: gather + softmax + aggregate ----------------
        with (
            tc.tile_pool(name="p1c", bufs=1) as c1,
            tc.tile_pool(name="p1g", bufs=2) as gp,
            tc.tile_pool(name="p1s", bufs=4) as sp,
            tc.tile_pool(name="p1m", bufs=3) as mp,
            tc.tile_pool(name="p1ps", bufs=2, space="PSUM") as pp1,
        ):
            iota_i = c1.tile([P, P], i32)
            nc.gpsimd.iota(iota_i[:], pattern=[[1, P]], base=0, channel_multiplier=0)
            iota_f = c1.tile([P, P], f32)
            nc.vector.tensor_copy(iota_f[:], iota_i[:])
            iota_bf = c1.tile([P, P], bf16)
            nc.vector.tensor_copy(iota_bf[:], iota_f[:])
            # iota column: value = partition index (for onehotT is_equal)
            iotac_i = c1.tile([P, 1], i32)
            nc.gpsimd.iota(iotac_i[:], pattern=[[1, 1]], base=0, channel_multiplier=1)
            iotac = c1.tile([P, 1], f32)
            nc.vector.tensor_copy(iotac[:], iotac_i[:])
            ones_r = c1.tile([1, P], bf16)
            nc.vector.memset(ones_r[:], 1.0)

            for w in range(nw):
                if w >= wlim:
                    osbz = mp.tile([P, C], f32, tag="osb", name=f"osbz{w}")
                    nc.vector.memset(osbz[:], 0.0)
                    rz = min(P, d_per_core - w * P)
                    nc.sync.dma_start(outd[w * P : w * P + rz, :], osbz[:rz, :])
                    continue
                tl, th = t_lo[w], t_hi[w]
                tt = tl + th
                nchunk = (tt + 3) // 4
                G = gp.tile([P, tmax * GCOLS], bf16, tag="G")
                idxt = mp.tile([P, smax], i16, tag="idx")
                nc.sync.dma_start(idxt[:], widx[w])
                dl = mp.tile([P, tmax], f32, tag="dl")
                nc.sync.dma_start(dl[:], wdl[w])
                dlr = mp.tile([1, tmax * P], bf16, tag="dlr")
                nc.sync.dma_start(dlr[:, : tt * P], wdlr[w, :, : tt * P])
                mk = mp.tile([P, 1], f32, tag="mk")
                nc.sync.dma_start(mk[:], wmask[w])
                if stage < 1:
                    continue
                # chunk gathers to <=1024 indices: a single SWDGE gather
                # larger than the 1024-descriptor ring hangs the device
                GCH = 8
                for base, ntl, tab in ((0, tl, hA_lo), (tl, th, hA_hi)):
                    for c0 in range(0, ntl, GCH):
                        cn = min(GCH, ntl - c0)
                        t0g = base + c0
                        nc.gpsimd.dma_gather(
                            G[:, t0g * GCOLS : (t0g + cn) * GCOLS].rearrange(
                                "p (t e) -> p t e", e=GCOLS
                            ),
                            tab[:],
                            idxt[:, 8 * t0g : 8 * (t0g + cn)],
                            cn * P,
                            cn * P,
                            GCOLS,
                        )
                gf = G[:].bitcast(f32).rearrange("p (t c) -> p t c", c=GCOLS // 2)

                def _dump(tag_w=w):
                    osbx = mp.tile([P, C], f32, tag="osb", name=f"osbd{tag_w}")
                    nc.vector.memset(osbx[:], 0.0)
                    rx = min(P, d_per_core - tag_w * P)
                    nc.sync.dma_start(outd[tag_w * P : tag_w * P + rx, :], osbx[:rx, :])

                if stage < 2:
                    osb0 = mp.tile([P, C], f32, tag="osb")
                    nc.vector.memset(osb0[:], 0.0)
                    nc.vector.tensor_copy(osb0[:, 0 : 2 * tt], gf[:, :tt, 0:2])
                    rows0 = min(P, d_per_core - w * P)
                    nc.sync.dma_start(outd[w * P : w * P + rows0, :], osb0[:rows0, :])
                    continue
                # advec[p] = a_d of window dst p, merged from the partition-
                # aligned self-loop rows of tile 0 (lo) / tile tl (hi)
                adv = mp.tile([P, 1], f32, tag="adv")
                nc.vector.tensor_tensor(
                    out=adv[:], in0=gf[:, 0, 1:2], in1=gf[:, tl, 1:2], op=OP.subtract
                )
                nc.vector.tensor_tensor(out=adv[:], in0=adv[:], in1=mk[:], op=OP.mult)
                nc.vector.tensor_tensor(
                    out=adv[:], in0=adv[:], in1=gf[:, tl, 1:2], op=OP.add
                )
                adv_b = mp.tile([P, 1], bf16, tag="advb")
                nc.vector.tensor_copy(adv_b[:], adv[:])
                if stage < 2.3:
                    _dump()
                    continue

                # per-edge a_d[dst]: onehotT (dst x edge) built from the row-
                # broadcast dstloc, then a small matmul with advec
                ebuf = mp.tile([P, tmax], f32, tag="e")
                for ch in range(nchunk):
                    t0 = ch * 4
                    t1 = min(t0 + 4, tt)
                    pdlb = pp1.tile([P, 4 * P], f32, tag="pdlb", bufs=2)
                    nc.tensor.matmul(
                        pdlb[:, : (t1 - t0) * P],
                        lhsT=ones_r[:],
                        rhs=dlr[:, t0 * P : t1 * P],
                        start=True,
                        stop=True,
                    )
                    dlb = sp.tile([P, 4 * P], bf16, tag="dlb")
                    nc.vector.tensor_copy(dlb[:, : (t1 - t0) * P], pdlb[:, : (t1 - t0) * P])
                    if stage < 2.5:
                        continue
                    pad4 = pp1.tile([P, 4], f32, tag="pad4", bufs=2)
                    for t in range(t0, t1):
                        oht = sp.tile([P, P], bf16, tag="oht")
                        nc.vector.tensor_scalar(
                            out=oht[:],
                            in0=dlb[:, (t - t0) * P : (t - t0 + 1) * P],
                            scalar1=iotac[:, 0:1],
                            scalar2=None,
                            op0=OP.is_equal,
                        )
                        nc.tensor.matmul(
                            pad4[:, t - t0 : t - t0 + 1],
                            lhsT=oht[:],
                            rhs=adv_b[:],
                            start=True,
                            stop=True,
                        )
                    # e = a_s[src] + a_d[dst]
                    nc.vector.tensor_tensor(
                        out=ebuf[:, t0:t1],
                        in0=gf[:, t0:t1, 0:1],
                        in1=pad4[:, : t1 - t0],
                        op=OP.add,
                    )
                if stage < 2.5:
                    _dump()
                    continue
                if stage < 2.7:
                    _dump()
                    continue
                e2 = mp.tile([P, tmax], f32, tag="e2")
                xb = mp.tile([P, tmax], f32, tag="xb")
                nc.vector.tensor_scalar(
                    out=e2[:, :tt], in0=ebuf[:, :tt], scalar1=0.2, scalar2=None,
                    op0=OP.mult,
                )
                nc.vector.tensor_tensor(
                    out=e2[:, :tt], in0=ebuf[:, :tt], in1=e2[:, :tt], op=OP.max
                )
                nc.scalar.activation(out=xb[:, :tt], in_=e2[:, :tt], func=AF.Exp)
                if stage < 3:
                    osb1 = mp.tile([P, C], f32, tag="osb")
                    nc.vector.memset(osb1[:], 0.0)
                    nc.vector.tensor_copy(osb1[:, 0:tmax], xb[:])
                    rows1 = min(P, d_per_core - w * P)
                    nc.sync.dma_start(outd[w * P : w * P + rows1, :], osb1[:rows1, :])
                    continue

                pw = pp1.tile([P, 1 + C], f32, tag="pw")
                for t in range(tt):
                    S = sp.tile([P, P], bf16, tag="S")
                    nc.vector.tensor_scalar(
                        out=S[:],
                        in0=iota_bf[:],
                        scalar1=dl[:, t : t + 1],
                        scalar2=xb[:, t : t + 1],
                        op0=OP.is_equal,
                        op1=OP.mult,
                    )
                    nc.tensor.matmul(
                        pw[:],
                        lhsT=S[:],
                        rhs=G[:, t * GCOLS + RHS_OFF : t * GCOLS + RHS_OFF + 1 + C],
                        start=(t == 0),
                        stop=(t == tt - 1),
                    )
                rows = min(P, d_per_core - w * P)
                rec = mp.tile([P, 1], f32, tag="rec")
                nc.vector.reciprocal(rec[:], pw[:, 0:1])
                osb = mp.tile([P, C], f32, tag="osb")
                nc.vector.tensor_scalar(
                    out=osb[:], in0=pw[:, 1 : 1 + C], scalar1=rec[:, 0:1],
                    scalar2=None, op0=OP.mult,
                )
                nc.sync.dma_start(outd[w * P : w * P + rows, :], osb[:rows, :])

    nc.compile()
    return nc


# --------------------------------------------------------------------------
# Entry point
# --------------------------------------------------------------------------
def _get_compiled(edge_index, n_nodes, n_cores, split):
    widx, wdl, wdlr, wmask, t_lo, t_hi, d_per_core, nw, tmax, smax = _prep_edges(
        edge_index, n_nodes, n_cores, split
    )
    key = (n_nodes, n_cores, split, tuple(t_lo), tuple(t_hi))
    if key not in _CACHE:
        _CACHE[key] = _build_nc(n_nodes, split, d_per_core, t_lo, t_hi, tmax, smax)
    return _CACHE[key], widx, wdl, wdlr, wmask, d_per_core


def kernel(x, edge_index, W, att_src, att_dst, bias):
    from concourse.bass_utils import run_bass_kernel_spmd

    x = np.asarray(x, dtype=np.float32)
    W = np.asarray(W, dtype=np.float32)
    n_nodes = x.shape[0]
    n_cores = 8
    split = 32768 if n_nodes > 32768 else max(P, (n_nodes // 2) // P * P)

    nc, widx, wdl, wdlr, wmask, d_per_core = _get_compiled(
        edge_index, n_nodes, n_cores, split
    )

    xT = np.ascontiguousarray(x.T)
    att2 = np.ascontiguousarray(
        np.stack(
            [np.asarray(att_src, np.float32), np.asarray(att_dst, np.float32)], axis=1
        )
    )
    bias2 = np.ascontiguousarray(np.asarray(bias, np.float32).reshape(1, C))

    in_maps = [
        {
            "xT": xT,
            "W": W,
            "att2": att2,
            "bias": bias2,
            "widx": widx[k],
            "wdl": wdl[k],
            "wdlr": wdlr[k][:, None, :],
            "wmask": wmask[k],
            "wdlr": wdlr[k][:, None, :],
            "wmask": wmask[k],
        }
        for k in range(n_cores)
    ]
    kw = {}
    if TRACE:
        kw = dict(trace=True)
        if TRACE_ALL_CORES:
            kw["trace_cores"] = list(range(n_cores))
    res = run_bass_kernel_spmd(nc, in_maps, list(range(n_cores)), **kw)
    out = np.concatenate([res.results[k]["out"] for k in range(n_cores)], axis=0)
    kernel.last_exec_time_ns = res.exec_time_ns
    kernel.last_mean_exec_time_ns = res.mean_exec_time_ns
    return out


kernel.last_exec_time_ns = None
kernel.last_mean_exec_time_ns = None


# --------------------------------------------------------------------------
# Timing helper (no NTFF hook in this environment): time repeated PJRT
# executions with device-resident inputs; subtract a trivial-kernel baseline.
# --------------------------------------------------------------------------
def make_runner(nc, in_maps, n_cores):
    import jax
    import jax.numpy as jnp
    from jax.sharding import Mesh, PartitionSpec
    from jax.experimental.shard_map import shard_map
    from concourse import bass2jax, mybir

    bass2jax.install_neuronx_cc_hook()
    partition_name = (
        nc.partition_id_tensor.name if nc.partition_id_tensor else None
    )
    in_names, out_names, out_avals, zero_outs = [], [], [], []
    for alloc in nc.m.functions[0].allocations:
        if not isinstance(alloc, mybir.MemoryLocationSet):
            continue
        name = alloc.memorylocations[0].name
        if alloc.kind == "ExternalInput":
            if name != partition_name:
                in_names.append(name)
        elif alloc.kind == "ExternalOutput":
            out_names.append(name)
            shape = tuple(alloc.tensor_shape)
            dtype = mybir.dt.np(alloc.dtype)
            out_avals.append(jax.core.ShapedArray(shape, dtype))
            zero_outs.append(np.zeros(shape, dtype))
    n_params = len(in_names)
    all_in_names = list(in_names) + list(out_names)
    if partition_name is not None:
        all_in_names.append(partition_name)

    def _body(*args):
        operands = list(args)
        if partition_name is not None:
            operands.append(bass2jax.partition_id_tensor())
        outs = bass2jax._bass_exec_p.bind(
            *operands,
            out_avals=tuple(out_avals),
            in_names=tuple(all_in_names),
            out_names=tuple(out_names),
            lowering_input_output_aliases=(),
            sim_require_finite=True,
            sim_require_nnan=True,
            nc=nc,
        )
        return tuple(outs)

    devices = jax.devices()[:n_cores]
    mesh = Mesh(np.asarray(devices), ("core",))
    in_specs = (PartitionSpec("core"),) * (n_params + len(out_names))
    out_specs = (PartitionSpec("core"),) * len(out_names)
    fn = jax.jit(
        shard_map(
            _body, mesh=mesh, in_specs=in_specs, out_specs=out_specs,
            check_rep=False,
        ),
        keep_unused=True,
    )
    concat_in = [
        np.concatenate([np.asarray(in_maps[c][nm]) for c in range(n_cores)], axis=0)
        for nm in in_names
    ]
    concat_zeros = [
        np.zeros((n_cores * z.shape[0], *z.shape[1:]), z.dtype) for z in zero_outs
    ]
    sharding = jax.sharding.NamedSharding(mesh, PartitionSpec("core"))
    dev_in = [jax.device_put(a, sharding) for a in concat_in + concat_zeros]

    def run():
        outs = fn(*dev_in)
        jax.block_until_ready(outs)
        return outs

    return run, out_names, out_avals


def timed_kernel(x, edge_index, W, att_src, att_dst, bias, iters=20):
    """Run like kernel() but also time steady-state executions."""
    import time as _time

    x = np.asarray(x, dtype=np.float32)
    W = np.asarray(W, dtype=np.float32)
    n_nodes = x.shape[0]
    n_cores = 8
    split = 32768 if n_nodes > 32768 else max(P, (n_nodes // 2) // P * P)
    nc, widx, wdl, wdlr, wmask, d_per_core = _get_compiled(
        edge_index, n_nodes, n_cores, split
    )
    xT = np.ascontiguousarray(x.T)
    att2 = np.ascontiguousarray(
        np.stack(
            [np.asarray(att_src, np.float32), np.asarray(att_dst, np.float32)],
            axis=1,
        )
    )
    bias2 = np.ascontiguousarray(np.asarray(bias, np.float32).reshape(1, C))
    in_maps = [
        {
            "xT": xT, "W": W, "att2": att2, "bias": bias2,
            "widx": widx[k], "wdl": wdl[k],
            "wdlr": wdlr[k][:, None, :], "wmask": wmask[k],
        }
        for k in range(n_cores)
    ]
    run, out_names, out_avals = make_runner(nc, in_maps, n_cores)
    outs = run()  # warmup / compile
    t0 = _time.time()
    for _ in range(iters):
        outs = run()
    dt = (_time.time() - t0) / iters
    oi = out_names.index("out")
    out = np.asarray(outs[oi]).reshape(n_cores, d_per_core, C).reshape(-1, C)
    return out, dt

